# revision 19
# baseline (speedup 1.0000x reference)
"""BianGua attention kernel for 8 TRN2 NeuronCores.

Sharding: 24 (batch, head) pairs -> core c handles batch b = c//4 and the
3 heads [3g, 3g+3) with g = c%4.  Each core computes q/k/v projections for
its heads, causal flash-style attention with the hexagram bias folded into
the QK matmul (augmented contraction dim 64+6=70), and its partial slice of
the output projection.  The host sums the 4 partial outputs per batch
(the tensor-parallel all-reduce done at gather time).

Softmax uses no max-subtraction: valid scores are in [-29, 42] for these
input statistics, so exp() stays comfortably inside fp32 range.

v3 design notes:
- sigmoid(lam) is folded into the hexagram weights on the HOST
  (hexgF = hexagrams * 2*sqrt(sigmoid(lam))), so the q-side and k-side
  hex rows of the augmented q/k tiles are identical.
- v blocks are 128 wide per head: cols 0:64 hold v, cols 64:128 hold a
  constant-ones block, so the PV matmul emits softmax row-sums already
  replicated over PSUM partitions 64:128.  Normalization is a single-
  instruction reciprocal_approx_fast (~51 ULP) plus one multiply.
- the causal mask inside diagonal 128x128 blocks is applied by an extra
  accumulating matmul (stationary -3e38 strictly-upper bf16 matrix,
  moving identity) instead of elementwise multiplies.
- x transposed arrives in 24 column-major chunks, issued from both the
  sync and scalar DMA queues, so query-block-0 projections and attention
  start while the rest of x is still in flight.  Projections for blocks
  1-3 are dribbled into the attention pipeline like the v projection.
"""

import numpy as np
import ml_dtypes
from contextlib import ExitStack

import concourse.bass as bass
import concourse.mybir as mybir
import concourse.tile as tile
from concourse import bacc
from concourse.bass import ts, ds
from concourse.bass_utils import run_bass_kernel_spmd

F32 = mybir.dt.float32
F32R = mybir.dt.float32r
BF16 = mybir.dt.bfloat16
F16 = mybir.dt.float16
AF = mybir.ActivationFunctionType
BF16NP = ml_dtypes.bfloat16

T = 2048
DM = 768
D = 64
NH = 3           # heads per core
QT = 512         # query tile width
NQT = T // QT    # 4
KCH = 128        # key chunk
NKC = T // KCH   # 16
KC6 = DM // 128  # 6 contraction chunks for projections
SM_SCALE = float(D) ** -0.5  # 0.125

_CACHED_NC = None


def _build():
    nc = bacc.Bacc("TRN2", debug=False, num_devices=8)

    xT = nc.dram_tensor("xT", [DM, T], F16, kind="ExternalInput").ap()
    hexT = nc.dram_tensor("hexT", [64, T], F16, kind="ExternalInput").ap()
    hexgF = nc.dram_tensor("hexgF", [64, 6], F16, kind="ExternalInput").ap()
    wqkT = nc.dram_tensor("wqkT", [DM, 384], F16, kind="ExternalInput").ap()
    wvT = nc.dram_tensor("wvT", [DM, 192], F16, kind="ExternalInput").ap()
    woT = nc.dram_tensor("woT", [256, DM], F16, kind="ExternalInput").ap()
    maskT = nc.dram_tensor("maskT", [128, 128], BF16,
                           kind="ExternalInput").ap()
    identb = nc.dram_tensor("identb", [128, 128], BF16,
                            kind="ExternalInput").ap()
    out = nc.dram_tensor("out", [T, DM], F16, kind="ExternalOutput").ap()

    with tile.TileContext(nc) as tc:
        with ExitStack() as ctx:
            sb1 = ctx.enter_context(tc.tile_pool(name="sb1", bufs=1))
            sbw = ctx.enter_context(tc.tile_pool(name="sbw", bufs=8))
            sbo = ctx.enter_context(tc.tile_pool(name="sbo", bufs=2))
            sbp = ctx.enter_context(tc.tile_pool(name="sbp", bufs=4))
            pp_st = ctx.enter_context(
                tc.tile_pool(name="pp_st", bufs=2, space="PSUM"))
            pp_op = ctx.enter_context(
                tc.tile_pool(name="pp_op", bufs=2, space="PSUM"))
            pp_mm = ctx.enter_context(
                tc.tile_pool(name="pp_mm", bufs=2, space="PSUM"))

            # ---- resident SBUF tiles ----
            hexgF_sb = sb1.tile([64, 6], F16, tag="hexgF")
            hexT_sb = sb1.tile([64, T], F16, tag="hexT")
            wqk_sb = sb1.tile([128, KC6, 384], F16, tag="wqk")
            wv_sb = sb1.tile([128, KC6, 192], F16, tag="wv")
            wo_sb = sb1.tile([128, 2, DM], F16, tag="wo")
            maskT_sb = sb1.tile([128, 128], BF16, tag="maskT")
            ident_sb = sb1.tile([128, 128], BF16, tag="ident")
            v_sb = sb1.tile([128, NKC, NH, 128], F32R, tag="v")
            outT_sb = sb1.tile([128, 2, T], F16, tag="outT")
            qaug = [sb1.tile([70, T], F32R, tag=f"qaug{h}", name=f"qaug{h}")
                    for h in range(NH)]
            kaug = [sb1.tile([70, T], F32R, tag=f"kaug{h}", name=f"kaug{h}")
                    for h in range(NH)]
            xT_sb = sb1.tile([128, KC6, T], F16, tag="xT")

            # ---- phase 0: DMAs, in consumption order.  x chunks are
            # column-major (all 6 contraction chunks of query block 0
            # first); cc 0/1 issue from the sync queue, cc 2/3 from the
            # scalar queue so trigger serialization halves. ----
            # sync queue, in consumption order: soft-hex inputs, block-0
            # projections, then the rest.  One trigger per x column block
            # (3D AP over the 6 contraction chunks) keeps the queue short.
            nc.sync.dma_start(hexgF_sb[:], hexgF)
            for cc in range(2):
                nc.sync.dma_start(hexT_sb[:, ts(cc, T // 2)],
                                  hexT[:, ts(cc, T // 2)])
            wqk_r = wqkT.rearrange("(o p) m -> p o m", p=128)
            nc.sync.dma_start(wqk_sb[:], wqk_r)
            xT_r = xT.rearrange("(o p) (c t) -> p o c t", p=128, c=NQT)
            xT_sbr = xT_sb[:].rearrange("p o (c t) -> p o c t", c=NQT)
            nc.sync.dma_start(xT_sbr[:, :, 0, :], xT_r[:, :, 0, :])
            wv_r = wvT.rearrange("(o p) m -> p o m", p=128)
            nc.sync.dma_start(wv_sb[:], wv_r)
            nc.sync.dma_start(maskT_sb[:], maskT)
            nc.sync.dma_start(ident_sb[:], identb)
            for cc in range(1, 4):
                nc.sync.dma_start(xT_sbr[:, :, cc, :], xT_r[:, :, cc, :])
            wo_r = woT.rearrange("(o p) n -> p o n", p=128)
            nc.sync.dma_start(wo_sb[:], wo_r)

            # constant-ones blocks of v (cols 64:128 of each head block)
            nc.vector.memset(v_sb[:, :, :, 64:128].bitcast(F32), 1.0)

            # ---- phase 1: soft-hex rows into aug tiles ----
            # kaug[0] gets the PSUM evacuations; replicas are engine copies
            # (vector/scalar) ordered by when each head first needs them.
            for nt in range(NQT):
                shp = pp_mm.tile([6, QT], F32, tag="mm", name="shp")
                nc.tensor.matmul(shp[:], hexgF_sb[:], hexT_sb[:, ts(nt, QT)],
                                 start=True, stop=True)
                nc.vector.tensor_copy(kaug[0][64:70, ts(nt, QT)], shp[:])
            nc.vector.tensor_copy(qaug[0][64:70, :], kaug[0][64:70, :])
            nc.scalar.copy(kaug[1][64:70, :], kaug[0][64:70, :])
            nc.vector.tensor_copy(qaug[1][64:70, :], kaug[0][64:70, :])
            nc.scalar.copy(kaug[2][64:70, :], kaug[0][64:70, :])
            nc.vector.tensor_copy(qaug[2][64:70, :], kaug[0][64:70, :])

            # ---- phase 2/3: projections.  Query-block-0 q/k and v chunks
            # 0-3 are emitted up front; everything else dribbles into the
            # attention pipeline one item per chunk-pair. ----
            # wqk rows: [qA qB | qC kA | kB kC] in groups of 128
            grp_dst = [(qaug[0], qaug[1]), (qaug[2], kaug[0]),
                       (kaug[1], kaug[2])]

            def make_p(grp, nt):
                def emit():
                    dA, dB = grp_dst[grp]
                    pj = pp_mm.tile([128, QT], F32, tag="mm", name="pj")
                    for kc in range(KC6):
                        nc.tensor.matmul(
                            pj[:], wqk_sb[:, kc, ts(grp, 128)],
                            xT_sb[:, kc, ts(nt, QT)],
                            start=(kc == 0), stop=(kc == KC6 - 1))
                    nc.scalar.copy(dA[0:64, ts(nt, QT)], pj[0:64, :])
                    nc.vector.tensor_copy(dB[0:64, ts(nt, QT)],
                                          pj[64:128, :])
                return emit

            def make_v(ti):
                def emit():
                    vp = pp_mm.tile([128, 192], F32, tag="mm", name="vp")
                    for kc in range(KC6):
                        nc.tensor.matmul(
                            vp[:], xT_sb[:, kc, ts(ti, 128)],
                            wv_sb[:, kc, :],
                            start=(kc == 0), stop=(kc == KC6 - 1))
                    vpr = vp[:].rearrange("p (h x) -> p h x", h=NH)
                    nc.vector.tensor_copy(v_sb[:, ti, :, 0:64], vpr)
                return emit

            for grp in range(3):
                make_p(grp, 0)()
            for ti in range(4):
                make_v(ti)()

            work_queue = []
            for nt in range(1, NQT):
                for grp in range(3):
                    work_queue.append(make_p(grp, nt))
                if nt < NQT - 1:
                    for ti in range(4 * nt, 4 * nt + 4):
                        work_queue.append(make_v(ti))
            for ti in range(12, 16):
                work_queue.append(make_v(ti))

            # ---- phase 4: attention (j-outer) with the output projection
            # for query block j-1 dribbled into j's pipeline ----
            out_r = out.rearrange("(n p) c -> p n c", p=128)
            pending = []   # [(op_tile, rec_sb, dst_ap)] normalizations

            def flush_pending():
                while pending:
                    op_t, recs, dst_ap = pending.pop(0)
                    for half in range(2):
                        nc.vector.tensor_mul(
                            dst_ap[:, ts(half, 256)],
                            op_t[0:64, ts(half, 256)], recs[half][:])

            os_tiles = {}

            def make_wo(ti):
                def emit():
                    gi = ti // 2
                    if ti % 2 == 0:
                        os_tiles[gi] = sbo.tile([128, 2, DM], F16, tag="os",
                                                name="os")
                    os_sb = os_tiles[gi]
                    for nh2 in range(2):
                        wop = pp_mm.tile([128, 384], F32, tag="mm",
                                         name="wop")
                        nc.tensor.matmul(
                            wop[:], outT_sb[:, 0, ts(ti, 128)],
                            wo_sb[:, 0, ts(nh2, 384)],
                            start=True, stop=False)
                        nc.tensor.matmul(
                            wop[:], outT_sb[0:64, 1, ts(ti, 128)],
                            wo_sb[0:64, 1, ts(nh2, 384)],
                            start=False, stop=True)
                        nc.vector.tensor_copy(
                            os_sb[:, ti % 2, ts(nh2, 384)], wop[:])
                    if ti % 2 == 1:
                        nc.sync.dma_start(
                            out_r[:, ds(2 * gi, 2), :], os_sb[:])
                return emit

            for j in range(NQT):
                for h in range(NH):
                    op = pp_op.tile([128, QT], F32, tag="op")
                    npair = 2 * j + 2
                    pends = []
                    for pi in range(npair):
                        # chunk pair (2*pi, 2*pi+1)
                        stp = pp_st.tile([128, 2, QT], F32, tag="st")
                        w0s = []
                        for s in range(2):
                            c = 2 * pi + s
                            r = c - 4 * j
                            w0 = KCH * r if r >= 0 else 0
                            w0s.append(w0)
                            nc.tensor.matmul(
                                stp[:, s, w0:QT],
                                kaug[h][0:70, ts(c, KCH)],
                                qaug[h][0:70, j * QT + w0: (j + 1) * QT],
                                start=True, stop=(r < 0))
                            if r >= 0:
                                # causal mask inside the diagonal block:
                                # accumulate -3e38 above the diagonal
                                nc.tensor.matmul(
                                    stp[:, s, w0:w0 + KCH],
                                    maskT_sb[:], ident_sb[:],
                                    start=False, stop=True)
                        if pi == 0:
                            flush_pending()
                        if work_queue:
                            work_queue.pop(0)()
                        p_sb = sbp.tile([128, 2, QT], F32R, tag="p")
                        wmin = min(w0s)
                        nc.scalar.activation(
                            p_sb[:, :, wmin:QT], stp[:, :, wmin:QT], AF.Exp,
                            scale=SM_SCALE)
                        pends.append((p_sb, pi, w0s))
                        if len(pends) > 2:
                            pp_t, ppi, pw0s = pends.pop(0)
                            for s in range(2):
                                c = 2 * ppi + s
                                nc.tensor.matmul(
                                    op[:, pw0s[s]:QT],
                                    v_sb[:, c, h, :],
                                    pp_t[:, s, pw0s[s]:QT],
                                    start=(c == 0), stop=False)
                    while pends:
                        pp_t, ppi, pw0s = pends.pop(0)
                        last = not pends
                        for s in range(2):
                            c = 2 * ppi + s
                            nc.tensor.matmul(
                                op[:, pw0s[s]:QT],
                                v_sb[:, c, h, :],
                                pp_t[:, s, pw0s[s]:QT],
                                start=(c == 0), stop=(last and s == 1))
                    # rows 64:128 of op hold the softmax row-sums already
                    # replicated across partitions (ones block of v).
                    # reciprocal_approx_fast needs full-width offset-0 APs,
                    # so stage each 256-wide half into its own tile first.
                    recs = []
                    for half in range(2):
                        tmp = sbw.tile([64, 256], F32, tag="tmp",
                                       name="tmp")
                        nc.vector.tensor_copy(tmp[:],
                                              op[64:128, ts(half, 256)])
                        rc = sbw.tile([64, 256], F32, tag="rec", name="rc")
                        nc.vector.reciprocal_approx_fast(rc[:], tmp[:])
                        recs.append(rc)
                    dst = outT_sb[64 * (h % 2): 64 * (h % 2) + 64, h // 2,
                                  ts(j, QT)]
                    pending.append((op, recs, dst))
                # all heads of block j done: finish normalizations, then
                # queue its output-projection chunks for block j+1's pipeline
                flush_pending()
                for ti in range(4 * j, 4 * j + 4):
                    work_queue.append(make_wo(ti))
            while work_queue:
                work_queue.pop(0)()

    nc.compile()
    return nc


def _prep_in_maps(inputs):
    x = np.asarray(inputs["x"], dtype=np.float32)
    hexw = np.asarray(inputs["hex_weights"], dtype=np.float32)
    Wq = np.asarray(inputs["Wq"], dtype=np.float32)
    Wk = np.asarray(inputs["Wk"], dtype=np.float32)
    Wv = np.asarray(inputs["Wv"], dtype=np.float32)
    Wo = np.asarray(inputs["Wo"], dtype=np.float32)
    lam = float(np.asarray(inputs["lam_logit"], dtype=np.float64))
    sig = 1.0 / (1.0 + np.exp(-lam))
    hexgF = np.ascontiguousarray(
        (np.asarray(inputs["hexagrams"], dtype=np.float64)
         * 2.0 * np.sqrt(sig)).astype(np.float16))
    maskT = np.triu(np.full((128, 128), -3.0e38, np.float32), 1)
    maskT = np.ascontiguousarray(maskT.astype(BF16NP))
    identb = np.ascontiguousarray(np.eye(128, dtype=np.float32)
                                  .astype(BF16NP))

    in_maps = []
    for c in range(8):
        b, g = c // 4, c % 4
        hs = slice(192 * g, 192 * (g + 1))
        xTn = np.ascontiguousarray(x[b].T.astype(np.float16))
        hexTn = np.ascontiguousarray(hexw[b].T.astype(np.float16))
        wqk = np.concatenate([Wq[hs], Wk[hs]], axis=0)      # [384, 768]
        wqkT = np.ascontiguousarray(wqk.T.astype(np.float16))  # [768, 384]
        wvT = np.ascontiguousarray(Wv[hs].T.astype(np.float16))  # [768, 192]
        woT = np.zeros((256, DM), np.float16)
        woT[:192] = Wo[:, hs].T                             # [192, 768]
        in_maps.append({
            "xT": xTn, "hexT": hexTn, "hexgF": hexgF, "wqkT": wqkT,
            "wvT": wvT, "woT": np.ascontiguousarray(woT),
            "maskT": maskT, "identb": identb,
        })
    return in_maps


LAST_RESULTS = None


def _run(inputs, **kwargs):
    global _CACHED_NC, LAST_RESULTS
    if _CACHED_NC is None:
        _CACHED_NC = _build()
    in_maps = _prep_in_maps(inputs)
    res = run_bass_kernel_spmd(_CACHED_NC, in_maps, core_ids=list(range(8)),
                               **kwargs)
    LAST_RESULTS = res
    outs = [r["out"].astype(np.float32) for r in res.results]
    y = np.empty((2, T, DM), np.float32)
    y[0] = outs[0] + outs[1] + outs[2] + outs[3]
    y[1] = outs[4] + outs[5] + outs[6] + outs[7]
    return y


def kernel(**inputs):
    return _run(inputs)


# revision 20
# speedup vs baseline: 1.0093x; 1.0093x over previous
"""BianGua attention kernel for 8 TRN2 NeuronCores.

Sharding: 24 (batch, head) pairs -> core c handles batch b = c//4 and the
3 heads [3g, 3g+3) with g = c%4.  Each core computes q/k/v projections for
its heads, causal flash-style attention with the hexagram bias folded into
the QK matmul (augmented contraction dim 64+6=70), and its partial slice of
the output projection.  The host sums the 4 partial outputs per batch
(the tensor-parallel all-reduce done at gather time).

Softmax uses no max-subtraction: valid scores are in [-29, 42] for these
input statistics, so exp() stays comfortably inside fp32 range.

v3 design notes:
- sigmoid(lam) is folded into the hexagram weights on the HOST
  (hexgF = hexagrams * 2*sqrt(sigmoid(lam))), so the q-side and k-side
  hex rows of the augmented q/k tiles are identical.
- v blocks are 128 wide per head: cols 0:64 hold v, cols 64:128 hold a
  constant-ones block, so the PV matmul emits softmax row-sums already
  replicated over PSUM partitions 64:128.  Normalization is a single-
  instruction reciprocal_approx_fast (~51 ULP) plus one multiply.
- the causal mask inside diagonal 128x128 blocks is applied by an extra
  accumulating matmul (stationary -3e38 strictly-upper bf16 matrix,
  moving identity) instead of elementwise multiplies.
- x transposed arrives in 24 column-major chunks, issued from both the
  sync and scalar DMA queues, so query-block-0 projections and attention
  start while the rest of x is still in flight.  Projections for blocks
  1-3 are dribbled into the attention pipeline like the v projection.
"""

import numpy as np
import ml_dtypes
from contextlib import ExitStack

import concourse.bass as bass
import concourse.mybir as mybir
import concourse.tile as tile
from concourse import bacc
from concourse.bass import ts, ds
from concourse.bass_utils import run_bass_kernel_spmd

F32 = mybir.dt.float32
F32R = mybir.dt.float32r
BF16 = mybir.dt.bfloat16
F16 = mybir.dt.float16
AF = mybir.ActivationFunctionType
BF16NP = ml_dtypes.bfloat16

T = 2048
DM = 768
D = 64
NH = 3           # heads per core
QT = 512         # query tile width
NQT = T // QT    # 4
KCH = 128        # key chunk
NKC = T // KCH   # 16
KC6 = DM // 128  # 6 contraction chunks for projections
SM_SCALE = float(D) ** -0.5  # 0.125

_CACHED_NC = None


def _build():
    nc = bacc.Bacc("TRN2", debug=False, num_devices=8)

    xT = nc.dram_tensor("xT", [DM, T], F16, kind="ExternalInput").ap()
    hexT = nc.dram_tensor("hexT", [64, T], F16, kind="ExternalInput").ap()
    hexgF = nc.dram_tensor("hexgF", [64, 6], F16, kind="ExternalInput").ap()
    wqkT = nc.dram_tensor("wqkT", [DM, 384], F16, kind="ExternalInput").ap()
    wvT = nc.dram_tensor("wvT", [DM, 192], F16, kind="ExternalInput").ap()
    woT = nc.dram_tensor("woT", [256, DM], F16, kind="ExternalInput").ap()
    maskT = nc.dram_tensor("maskT", [128, 128], BF16,
                           kind="ExternalInput").ap()
    identb = nc.dram_tensor("identb", [128, 128], BF16,
                            kind="ExternalInput").ap()
    out = nc.dram_tensor("out", [T, DM], F16, kind="ExternalOutput").ap()

    with tile.TileContext(nc) as tc:
        with ExitStack() as ctx:
            sb1 = ctx.enter_context(tc.tile_pool(name="sb1", bufs=1))
            sbw = ctx.enter_context(tc.tile_pool(name="sbw", bufs=8))
            sbo = ctx.enter_context(tc.tile_pool(name="sbo", bufs=2))
            sbp = ctx.enter_context(tc.tile_pool(name="sbp", bufs=4))
            pp_st = ctx.enter_context(
                tc.tile_pool(name="pp_st", bufs=2, space="PSUM"))
            pp_op = ctx.enter_context(
                tc.tile_pool(name="pp_op", bufs=2, space="PSUM"))
            pp_mm = ctx.enter_context(
                tc.tile_pool(name="pp_mm", bufs=2, space="PSUM"))

            # ---- resident SBUF tiles ----
            hexgF_sb = sb1.tile([64, 6], F16, tag="hexgF")
            hexT_sb = sb1.tile([64, T], F16, tag="hexT")
            wqk_sb = sb1.tile([128, KC6, 384], F16, tag="wqk")
            wv_sb = sb1.tile([128, KC6, 192], F16, tag="wv")
            wo_sb = sb1.tile([128, 2, DM], F16, tag="wo")
            maskT_sb = sb1.tile([128, 128], BF16, tag="maskT")
            ident_sb = sb1.tile([128, 128], BF16, tag="ident")
            v_sb = sb1.tile([128, NKC, NH, 128], F32R, tag="v")
            outT_sb = sb1.tile([128, 2, T], F16, tag="outT")
            qaug = [sb1.tile([70, T], F32R, tag=f"qaug{h}", name=f"qaug{h}")
                    for h in range(NH)]
            kaug = [sb1.tile([70, T], F32R, tag=f"kaug{h}", name=f"kaug{h}")
                    for h in range(NH)]
            xT_sb = sb1.tile([128, KC6, T], F16, tag="xT")

            # ---- phase 0: DMAs, in consumption order.  x chunks are
            # column-major (all 6 contraction chunks of query block 0
            # first); cc 0/1 issue from the sync queue, cc 2/3 from the
            # scalar queue so trigger serialization halves. ----
            # sync queue, in consumption order: soft-hex inputs, block-0
            # projections, then the rest.  One trigger per x column block
            # (3D AP over the 6 contraction chunks) keeps the queue short.
            nc.sync.dma_start(hexgF_sb[:], hexgF)
            for cc in range(2):
                nc.sync.dma_start(hexT_sb[:, ts(cc, T // 2)],
                                  hexT[:, ts(cc, T // 2)])
            wqk_r = wqkT.rearrange("(o p) m -> p o m", p=128)
            nc.sync.dma_start(wqk_sb[:], wqk_r)
            xT_r = xT.rearrange("(o p) (c t) -> p o c t", p=128, c=NQT)
            xT_sbr = xT_sb[:].rearrange("p o (c t) -> p o c t", c=NQT)
            for kc in range(KC6):
                nc.sync.dma_start(xT_sbr[:, kc, 0, :], xT_r[:, kc, 0, :])
            wv_r = wvT.rearrange("(o p) m -> p o m", p=128)
            nc.sync.dma_start(wv_sb[:], wv_r)
            nc.sync.dma_start(maskT_sb[:], maskT)
            nc.sync.dma_start(ident_sb[:], identb)
            for cc in range(1, 4):
                for kc in range(KC6):
                    nc.sync.dma_start(xT_sbr[:, kc, cc, :],
                                      xT_r[:, kc, cc, :])
            wo_r = woT.rearrange("(o p) n -> p o n", p=128)
            nc.sync.dma_start(wo_sb[:], wo_r)

            # constant-ones blocks of v (cols 64:128 of each head block)
            nc.vector.memset(v_sb[:, :, :, 64:128].bitcast(F32), 1.0)

            # ---- phase 1: soft-hex rows into aug tiles ----
            # kaug[0] gets the PSUM evacuations; replicas are engine copies
            # (vector/scalar) ordered by when each head first needs them.
            for nt in range(NQT):
                shp = pp_mm.tile([6, QT], F32, tag="mm", name="shp")
                nc.tensor.matmul(shp[:], hexgF_sb[:], hexT_sb[:, ts(nt, QT)],
                                 start=True, stop=True)
                nc.vector.tensor_copy(kaug[0][64:70, ts(nt, QT)], shp[:])
            nc.vector.tensor_copy(qaug[0][64:70, :], kaug[0][64:70, :])
            nc.scalar.copy(kaug[1][64:70, :], kaug[0][64:70, :])
            nc.vector.tensor_copy(qaug[1][64:70, :], kaug[0][64:70, :])
            nc.scalar.copy(kaug[2][64:70, :], kaug[0][64:70, :])
            nc.vector.tensor_copy(qaug[2][64:70, :], kaug[0][64:70, :])

            # ---- phase 2/3: projections.  Query-block-0 q/k and v chunks
            # 0-3 are emitted up front; everything else dribbles into the
            # attention pipeline one item per chunk-pair. ----
            # wqk rows: [qA qB | qC kA | kB kC] in groups of 128
            grp_dst = [(qaug[0], qaug[1]), (qaug[2], kaug[0]),
                       (kaug[1], kaug[2])]

            def make_p(grp, nt):
                def emit():
                    dA, dB = grp_dst[grp]
                    pj = pp_mm.tile([128, QT], F32, tag="mm", name="pj")
                    for kc in range(KC6):
                        nc.tensor.matmul(
                            pj[:], wqk_sb[:, kc, ts(grp, 128)],
                            xT_sb[:, kc, ts(nt, QT)],
                            start=(kc == 0), stop=(kc == KC6 - 1))
                    nc.scalar.copy(dA[0:64, ts(nt, QT)], pj[0:64, :])
                    nc.vector.tensor_copy(dB[0:64, ts(nt, QT)],
                                          pj[64:128, :])
                return emit

            def make_v(ti):
                def emit():
                    vp = pp_mm.tile([128, 192], F32, tag="mm", name="vp")
                    for kc in range(KC6):
                        nc.tensor.matmul(
                            vp[:], xT_sb[:, kc, ts(ti, 128)],
                            wv_sb[:, kc, :],
                            start=(kc == 0), stop=(kc == KC6 - 1))
                    vpr = vp[:].rearrange("p (h x) -> p h x", h=NH)
                    nc.vector.tensor_copy(v_sb[:, ti, :, 0:64], vpr)
                return emit

            for grp in range(3):
                make_p(grp, 0)()
            for ti in range(4):
                make_v(ti)()

            work_queue = []
            for nt in range(1, NQT):
                for grp in range(3):
                    work_queue.append(make_p(grp, nt))
                if nt < NQT - 1:
                    for ti in range(4 * nt, 4 * nt + 4):
                        work_queue.append(make_v(ti))
            for ti in range(12, 16):
                work_queue.append(make_v(ti))

            # ---- phase 4: attention (j-outer) with the output projection
            # for query block j-1 dribbled into j's pipeline ----
            out_r = out.rearrange("(n p) c -> p n c", p=128)
            pending = []   # [(op_tile, rec_sb, dst_ap)] normalizations

            def flush_pending():
                while pending:
                    op_t, recs, dst_ap = pending.pop(0)
                    for half in range(2):
                        nc.vector.tensor_mul(
                            dst_ap[:, ts(half, 256)],
                            op_t[0:64, ts(half, 256)], recs[half][:])

            os_tiles = {}

            def make_wo(ti):
                def emit():
                    gi = ti // 2
                    if ti % 2 == 0:
                        os_tiles[gi] = sbo.tile([128, 2, DM], F16, tag="os",
                                                name="os")
                    os_sb = os_tiles[gi]
                    for nh2 in range(2):
                        wop = pp_mm.tile([128, 384], F32, tag="mm",
                                         name="wop")
                        nc.tensor.matmul(
                            wop[:], outT_sb[:, 0, ts(ti, 128)],
                            wo_sb[:, 0, ts(nh2, 384)],
                            start=True, stop=False)
                        nc.tensor.matmul(
                            wop[:], outT_sb[0:64, 1, ts(ti, 128)],
                            wo_sb[0:64, 1, ts(nh2, 384)],
                            start=False, stop=True)
                        nc.vector.tensor_copy(
                            os_sb[:, ti % 2, ts(nh2, 384)], wop[:])
                    if ti % 2 == 1:
                        nc.sync.dma_start(
                            out_r[:, ds(2 * gi, 2), :], os_sb[:])
                return emit

            for j in range(NQT):
                for h in range(NH):
                    op = pp_op.tile([128, QT], F32, tag="op")
                    npair = 2 * j + 2
                    pends = []
                    for pi in range(npair):
                        # chunk pair (2*pi, 2*pi+1)
                        stp = pp_st.tile([128, 2, QT], F32, tag="st")
                        w0s = []
                        for s in range(2):
                            c = 2 * pi + s
                            r = c - 4 * j
                            w0 = KCH * r if r >= 0 else 0
                            w0s.append(w0)
                            nc.tensor.matmul(
                                stp[:, s, w0:QT],
                                kaug[h][0:70, ts(c, KCH)],
                                qaug[h][0:70, j * QT + w0: (j + 1) * QT],
                                start=True, stop=(r < 0))
                            if r >= 0:
                                # causal mask inside the diagonal block:
                                # accumulate -3e38 above the diagonal
                                nc.tensor.matmul(
                                    stp[:, s, w0:w0 + KCH],
                                    maskT_sb[:], ident_sb[:],
                                    start=False, stop=True)
                        if pi == 0:
                            flush_pending()
                        if work_queue:
                            work_queue.pop(0)()
                        p_sb = sbp.tile([128, 2, QT], F32R, tag="p")
                        wmin = min(w0s)
                        nc.scalar.activation(
                            p_sb[:, :, wmin:QT], stp[:, :, wmin:QT], AF.Exp,
                            scale=SM_SCALE)
                        pends.append((p_sb, pi, w0s))
                        if len(pends) > 2:
                            pp_t, ppi, pw0s = pends.pop(0)
                            for s in range(2):
                                c = 2 * ppi + s
                                nc.tensor.matmul(
                                    op[:, pw0s[s]:QT],
                                    v_sb[:, c, h, :],
                                    pp_t[:, s, pw0s[s]:QT],
                                    start=(c == 0), stop=False)
                    while pends:
                        pp_t, ppi, pw0s = pends.pop(0)
                        last = not pends
                        for s in range(2):
                            c = 2 * ppi + s
                            nc.tensor.matmul(
                                op[:, pw0s[s]:QT],
                                v_sb[:, c, h, :],
                                pp_t[:, s, pw0s[s]:QT],
                                start=(c == 0), stop=(last and s == 1))
                    # rows 64:128 of op hold the softmax row-sums already
                    # replicated across partitions (ones block of v).
                    # reciprocal_approx_fast needs full-width offset-0 APs,
                    # so stage each 256-wide half into its own tile first.
                    recs = []
                    for half in range(2):
                        tmp = sbw.tile([64, 256], F32, tag="tmp",
                                       name="tmp")
                        nc.vector.tensor_copy(tmp[:],
                                              op[64:128, ts(half, 256)])
                        rc = sbw.tile([64, 256], F32, tag="rec", name="rc")
                        nc.vector.reciprocal_approx_fast(rc[:], tmp[:])
                        recs.append(rc)
                    dst = outT_sb[64 * (h % 2): 64 * (h % 2) + 64, h // 2,
                                  ts(j, QT)]
                    pending.append((op, recs, dst))
                # all heads of block j done: finish normalizations, then
                # queue its output-projection chunks for block j+1's pipeline
                flush_pending()
                for ti in range(4 * j, 4 * j + 4):
                    work_queue.append(make_wo(ti))
            while work_queue:
                work_queue.pop(0)()

    nc.compile()
    return nc


def _prep_in_maps(inputs):
    x = np.asarray(inputs["x"], dtype=np.float32)
    hexw = np.asarray(inputs["hex_weights"], dtype=np.float32)
    Wq = np.asarray(inputs["Wq"], dtype=np.float32)
    Wk = np.asarray(inputs["Wk"], dtype=np.float32)
    Wv = np.asarray(inputs["Wv"], dtype=np.float32)
    Wo = np.asarray(inputs["Wo"], dtype=np.float32)
    lam = float(np.asarray(inputs["lam_logit"], dtype=np.float64))
    sig = 1.0 / (1.0 + np.exp(-lam))
    hexgF = np.ascontiguousarray(
        (np.asarray(inputs["hexagrams"], dtype=np.float64)
         * 2.0 * np.sqrt(sig)).astype(np.float16))
    maskT = np.triu(np.full((128, 128), -3.0e38, np.float32), 1)
    maskT = np.ascontiguousarray(maskT.astype(BF16NP))
    identb = np.ascontiguousarray(np.eye(128, dtype=np.float32)
                                  .astype(BF16NP))

    in_maps = []
    for c in range(8):
        b, g = c // 4, c % 4
        hs = slice(192 * g, 192 * (g + 1))
        xTn = np.ascontiguousarray(x[b].T.astype(np.float16))
        hexTn = np.ascontiguousarray(hexw[b].T.astype(np.float16))
        wqk = np.concatenate([Wq[hs], Wk[hs]], axis=0)      # [384, 768]
        wqkT = np.ascontiguousarray(wqk.T.astype(np.float16))  # [768, 384]
        wvT = np.ascontiguousarray(Wv[hs].T.astype(np.float16))  # [768, 192]
        woT = np.zeros((256, DM), np.float16)
        woT[:192] = Wo[:, hs].T                             # [192, 768]
        in_maps.append({
            "xT": xTn, "hexT": hexTn, "hexgF": hexgF, "wqkT": wqkT,
            "wvT": wvT, "woT": np.ascontiguousarray(woT),
            "maskT": maskT, "identb": identb,
        })
    return in_maps


LAST_RESULTS = None


def _run(inputs, **kwargs):
    global _CACHED_NC, LAST_RESULTS
    if _CACHED_NC is None:
        _CACHED_NC = _build()
    in_maps = _prep_in_maps(inputs)
    res = run_bass_kernel_spmd(_CACHED_NC, in_maps, core_ids=list(range(8)),
                               **kwargs)
    LAST_RESULTS = res
    outs = [r["out"].astype(np.float32) for r in res.results]
    y = np.empty((2, T, DM), np.float32)
    y[0] = outs[0] + outs[1] + outs[2] + outs[3]
    y[1] = outs[4] + outs[5] + outs[6] + outs[7]
    return y


def kernel(**inputs):
    return _run(inputs)


# revision 21
# speedup vs baseline: 1.0208x; 1.0114x over previous
"""BianGua attention kernel for 8 TRN2 NeuronCores.

Sharding: 24 (batch, head) pairs -> core c handles batch b = c//4 and the
3 heads [3g, 3g+3) with g = c%4.  Each core computes q/k/v projections for
its heads, causal flash-style attention with the hexagram bias folded into
the QK matmul (augmented contraction dim 64+6=70), and its partial slice of
the output projection.  The host sums the 4 partial outputs per batch
(the tensor-parallel all-reduce done at gather time).

Softmax uses no max-subtraction: valid scores are in [-29, 42] for these
input statistics, so exp() stays comfortably inside fp32 range.

v3 design notes:
- sigmoid(lam) is folded into the hexagram weights on the HOST
  (hexgF = hexagrams * 2*sqrt(sigmoid(lam))), so the q-side and k-side
  hex rows of the augmented q/k tiles are identical.
- v blocks are 128 wide per head: cols 0:64 hold v, cols 64:128 hold a
  constant-ones block, so the PV matmul emits softmax row-sums already
  replicated over PSUM partitions 64:128.  Normalization is a single-
  instruction reciprocal_approx_fast (~51 ULP) plus one multiply.
- the causal mask inside diagonal 128x128 blocks is applied by an extra
  accumulating matmul (stationary -3e38 strictly-upper bf16 matrix,
  moving identity) instead of elementwise multiplies.
- x transposed arrives in 24 column-major chunks, issued from both the
  sync and scalar DMA queues, so query-block-0 projections and attention
  start while the rest of x is still in flight.  Projections for blocks
  1-3 are dribbled into the attention pipeline like the v projection.
"""

import numpy as np
import ml_dtypes
from contextlib import ExitStack

import concourse.bass as bass
import concourse.mybir as mybir
import concourse.tile as tile
from concourse import bacc
from concourse.bass import ts, ds
from concourse.bass_utils import run_bass_kernel_spmd

F32 = mybir.dt.float32
F32R = mybir.dt.float32r
BF16 = mybir.dt.bfloat16
F16 = mybir.dt.float16
AF = mybir.ActivationFunctionType
BF16NP = ml_dtypes.bfloat16

T = 2048
DM = 768
D = 64
NH = 3           # heads per core
QT = 512         # query tile width
NQT = T // QT    # 4
KCH = 128        # key chunk
NKC = T // KCH   # 16
KC6 = DM // 128  # 6 contraction chunks for projections
SM_SCALE = float(D) ** -0.5  # 0.125

_CACHED_NC = None


def _build():
    nc = bacc.Bacc("TRN2", debug=False, num_devices=8)

    xT = nc.dram_tensor("xT", [DM, T], F16, kind="ExternalInput").ap()
    hexT = nc.dram_tensor("hexT", [64, T], F16, kind="ExternalInput").ap()
    hexgF = nc.dram_tensor("hexgF", [64, 6], F16, kind="ExternalInput").ap()
    wqkT = nc.dram_tensor("wqkT", [DM, 384], F16, kind="ExternalInput").ap()
    wvT = nc.dram_tensor("wvT", [DM, 192], F16, kind="ExternalInput").ap()
    woT = nc.dram_tensor("woT", [256, DM], F16, kind="ExternalInput").ap()
    maskT = nc.dram_tensor("maskT", [128, 128], BF16,
                           kind="ExternalInput").ap()
    identb = nc.dram_tensor("identb", [128, 256], BF16,
                            kind="ExternalInput").ap()
    out = nc.dram_tensor("out", [T, DM], F16, kind="ExternalOutput").ap()

    with tile.TileContext(nc) as tc:
        with ExitStack() as ctx:
            sb1 = ctx.enter_context(tc.tile_pool(name="sb1", bufs=1))
            sbw = ctx.enter_context(tc.tile_pool(name="sbw", bufs=8))
            sbo = ctx.enter_context(tc.tile_pool(name="sbo", bufs=2))
            sbp = ctx.enter_context(tc.tile_pool(name="sbp", bufs=4))
            pp_st = ctx.enter_context(
                tc.tile_pool(name="pp_st", bufs=2, space="PSUM"))
            pp_op = ctx.enter_context(
                tc.tile_pool(name="pp_op", bufs=2, space="PSUM"))
            pp_mm = ctx.enter_context(
                tc.tile_pool(name="pp_mm", bufs=2, space="PSUM"))

            # ---- resident SBUF tiles ----
            hexgF_sb = sb1.tile([64, 6], F16, tag="hexgF")
            hexT_sb = sb1.tile([64, T], F16, tag="hexT")
            wqk_sb = sb1.tile([128, KC6, 384], F16, tag="wqk")
            wv_sb = sb1.tile([128, KC6, 192], F16, tag="wv")
            wo_sb = sb1.tile([128, 2, DM], F16, tag="wo")
            maskT_sb = sb1.tile([128, 128], BF16, tag="maskT")
            ident_sb = sb1.tile([128, 256], BF16, tag="ident")
            v_sb = sb1.tile([128, NKC, NH, 128], F32R, tag="v")
            outT_sb = sb1.tile([128, 2, T], F16, tag="outT")
            qaug = [sb1.tile([70, T], F32R, tag=f"qaug{h}", name=f"qaug{h}")
                    for h in range(NH)]
            kaug = [sb1.tile([70, T], F32R, tag=f"kaug{h}", name=f"kaug{h}")
                    for h in range(NH)]
            xT_sb = sb1.tile([128, KC6, T], F16, tag="xT")

            # ---- phase 0: DMAs, in consumption order.  x chunks are
            # column-major (all 6 contraction chunks of query block 0
            # first); cc 0/1 issue from the sync queue, cc 2/3 from the
            # scalar queue so trigger serialization halves. ----
            # sync queue, in consumption order: soft-hex inputs, block-0
            # projections, then the rest.  One trigger per x column block
            # (3D AP over the 6 contraction chunks) keeps the queue short.
            nc.sync.dma_start(hexgF_sb[:], hexgF)
            for cc in range(2):
                nc.sync.dma_start(hexT_sb[:, ts(cc, T // 2)],
                                  hexT[:, ts(cc, T // 2)])
            wqk_r = wqkT.rearrange("(o p) m -> p o m", p=128)
            nc.sync.dma_start(wqk_sb[:], wqk_r)
            xT_r = xT.rearrange("(o p) (c t) -> p o c t", p=128, c=NQT)
            xT_sbr = xT_sb[:].rearrange("p o (c t) -> p o c t", c=NQT)
            for kc in range(KC6):
                nc.sync.dma_start(xT_sbr[:, kc, 0, :], xT_r[:, kc, 0, :])
            wv_r = wvT.rearrange("(o p) m -> p o m", p=128)
            nc.sync.dma_start(wv_sb[:], wv_r)
            nc.sync.dma_start(maskT_sb[:], maskT)
            nc.sync.dma_start(ident_sb[:], identb)
            for cc in range(1, 4):
                for kc in range(KC6):
                    nc.sync.dma_start(xT_sbr[:, kc, cc, :],
                                      xT_r[:, kc, cc, :])
            wo_r = woT.rearrange("(o p) n -> p o n", p=128)
            nc.sync.dma_start(wo_sb[:], wo_r)

            # constant-ones blocks of v (cols 64:128 of each head block)
            nc.vector.memset(v_sb[:, :, :, 64:128].bitcast(F32), 1.0)

            # ---- phase 1: soft-hex rows into aug tiles ----
            # kaug[0] gets the PSUM evacuations; replicas are engine copies
            # (vector/scalar) ordered by when each head first needs them.
            for nt in range(NQT):
                shp = pp_mm.tile([6, QT], F32, tag="mm", name="shp")
                nc.tensor.matmul(shp[:], hexgF_sb[:], hexT_sb[:, ts(nt, QT)],
                                 start=True, stop=True)
                nc.vector.tensor_copy(kaug[0][64:70, ts(nt, QT)], shp[:])
            nc.vector.tensor_copy(qaug[0][64:70, :], kaug[0][64:70, :])
            nc.scalar.copy(kaug[1][64:70, :], kaug[0][64:70, :])
            nc.vector.tensor_copy(qaug[1][64:70, :], kaug[0][64:70, :])
            nc.scalar.copy(kaug[2][64:70, :], kaug[0][64:70, :])
            nc.vector.tensor_copy(qaug[2][64:70, :], kaug[0][64:70, :])

            # ---- phase 2/3: projections.  Query-block-0 q/k and v chunks
            # 0-3 are emitted up front; everything else dribbles into the
            # attention pipeline one item per chunk-pair. ----
            # wqk rows: [qA qB | qC kA | kB kC] in groups of 128
            grp_dst = [(qaug[0], qaug[1]), (qaug[2], kaug[0]),
                       (kaug[1], kaug[2])]

            def make_p(grp, nt):
                def emit():
                    dA, dB = grp_dst[grp]
                    pj = pp_mm.tile([128, QT], F32, tag="mm", name="pj")
                    for kc in range(KC6):
                        nc.tensor.matmul(
                            pj[:], wqk_sb[:, kc, ts(grp, 128)],
                            xT_sb[:, kc, ts(nt, QT)],
                            start=(kc == 0), stop=(kc == KC6 - 1))
                    nc.scalar.copy(dA[0:64, ts(nt, QT)], pj[0:64, :])
                    nc.vector.tensor_copy(dB[0:64, ts(nt, QT)],
                                          pj[64:128, :])
                return emit

            def make_v(ti):
                def emit():
                    vp = pp_mm.tile([128, 192], F32, tag="mm", name="vp")
                    for kc in range(KC6):
                        nc.tensor.matmul(
                            vp[:], xT_sb[:, kc, ts(ti, 128)],
                            wv_sb[:, kc, :],
                            start=(kc == 0), stop=(kc == KC6 - 1))
                    vpr = vp[:].rearrange("p (h x) -> p h x", h=NH)
                    nc.vector.tensor_copy(v_sb[:, ti, :, 0:64], vpr)
                return emit

            for grp in range(3):
                make_p(grp, 0)()
            for ti in range(4):
                make_v(ti)()

            work_queue = []
            for nt in range(1, NQT):
                for grp in range(3):
                    work_queue.append(make_p(grp, nt))
                if nt < NQT - 1:
                    for ti in range(4 * nt, 4 * nt + 4):
                        work_queue.append(make_v(ti))
            for ti in range(12, 16):
                work_queue.append(make_v(ti))

            # ---- phase 4: attention (j-outer) with the output projection
            # for query block j-1 dribbled into j's pipeline ----
            out_r = out.rearrange("(n p) c -> p n c", p=128)
            pending = []   # [(op_tile, rec_sb, dst_ap)] normalizations

            def flush_pending():
                while pending:
                    op_t, recs, dst_ap = pending.pop(0)
                    for half in range(2):
                        nc.vector.tensor_mul(
                            dst_ap[:, ts(half, 256)],
                            op_t[0:64, ts(half, 256)], recs[half][:])

            os_tiles = {}

            def make_wo(ti):
                def emit():
                    gi = ti // 2
                    if ti % 2 == 0:
                        os_tiles[gi] = sbo.tile([128, 2, DM], F16, tag="os",
                                                name="os")
                    os_sb = os_tiles[gi]
                    for nh2 in range(2):
                        wop = pp_mm.tile([128, 384], F32, tag="mm",
                                         name="wop")
                        nc.tensor.matmul(
                            wop[:], outT_sb[:, 0, ts(ti, 128)],
                            wo_sb[:, 0, ts(nh2, 384)],
                            start=True, stop=False)
                        nc.tensor.matmul(
                            wop[:], outT_sb[0:64, 1, ts(ti, 128)],
                            wo_sb[0:64, 1, ts(nh2, 384)],
                            start=False, stop=True)
                        nc.vector.tensor_copy(
                            os_sb[:, ti % 2, ts(nh2, 384)], wop[:])
                    if ti % 2 == 1:
                        nc.sync.dma_start(
                            out_r[:, ds(2 * gi, 2), :], os_sb[:])
                return emit

            for j in range(NQT):
                for h in range(NH):
                    op = pp_op.tile([128, QT], F32, tag="op")
                    npair = 2 * j + 2
                    pends = []
                    for pi in range(npair):
                        # chunk pair (2*pi, 2*pi+1)
                        stp = pp_st.tile([128, 2, QT], F32, tag="st")
                        w0s = []
                        diag = 2 * pi >= 4 * j
                        for s in range(2):
                            c = 2 * pi + s
                            r = c - 4 * j
                            w0 = KCH * r if r >= 0 else 0
                            w0s.append(w0)
                            # f32r matmuls run 4x slower below 256 free
                            # cols; widen the last diagonal sliver instead
                            qw = min(w0, QT - 256) if r >= 0 else 0
                            nc.tensor.matmul(
                                stp[:, s, qw:QT],
                                kaug[h][0:70, ts(c, KCH)],
                                qaug[h][0:70, j * QT + qw: (j + 1) * QT],
                                start=True, stop=(r < 0),
                                skip_group_check=(r >= 0))
                        if diag:
                            # causal mask for both chunks of the pair in
                            # one matmul: staggered output AP (s-stride
                            # 512 + 128-col shift) x [I|I] moving operand
                            base = stp[:, 0, w0s[0]:w0s[0] + KCH]
                            mask_ap = bass.AP(
                                tensor=base.tensor, offset=base.offset,
                                ap=[list(base.ap[0]), [QT + KCH, 2],
                                    [1, KCH]])
                            nc.tensor.matmul(
                                mask_ap, maskT_sb[:], ident_sb[:],
                                start=False, stop=True,
                                skip_group_check=True)
                        if pi == 0:
                            flush_pending()
                        if work_queue:
                            work_queue.pop(0)()
                        p_sb = sbp.tile([128, 2, QT], F32R, tag="p")
                        wmin = min(w0s)
                        nc.scalar.activation(
                            p_sb[:, :, wmin:QT], stp[:, :, wmin:QT], AF.Exp,
                            scale=SM_SCALE)
                        pends.append((p_sb, pi, w0s))
                        if len(pends) > 2:
                            pp_t, ppi, pw0s = pends.pop(0)
                            for s in range(2):
                                c = 2 * ppi + s
                                nc.tensor.matmul(
                                    op[:, pw0s[s]:QT],
                                    v_sb[:, c, h, :],
                                    pp_t[:, s, pw0s[s]:QT],
                                    start=(c == 0), stop=False)
                    while pends:
                        pp_t, ppi, pw0s = pends.pop(0)
                        last = not pends
                        for s in range(2):
                            c = 2 * ppi + s
                            nc.tensor.matmul(
                                op[:, pw0s[s]:QT],
                                v_sb[:, c, h, :],
                                pp_t[:, s, pw0s[s]:QT],
                                start=(c == 0), stop=(last and s == 1))
                    # rows 64:128 of op hold the softmax row-sums already
                    # replicated across partitions (ones block of v).
                    # reciprocal_approx_fast needs full-width offset-0 APs,
                    # so stage each 256-wide half into its own tile first.
                    recs = []
                    for half in range(2):
                        tmp = sbw.tile([64, 256], F32, tag="tmp",
                                       name="tmp")
                        nc.vector.tensor_copy(tmp[:],
                                              op[64:128, ts(half, 256)])
                        rc = sbw.tile([64, 256], F32, tag="rec", name="rc")
                        nc.vector.reciprocal_approx_fast(rc[:], tmp[:])
                        recs.append(rc)
                    dst = outT_sb[64 * (h % 2): 64 * (h % 2) + 64, h // 2,
                                  ts(j, QT)]
                    pending.append((op, recs, dst))
                # all heads of block j done: finish normalizations, then
                # queue its output-projection chunks for block j+1's pipeline
                flush_pending()
                for ti in range(4 * j, 4 * j + 4):
                    work_queue.append(make_wo(ti))
            while work_queue:
                work_queue.pop(0)()

    nc.compile()
    return nc


def _prep_in_maps(inputs):
    x = np.asarray(inputs["x"], dtype=np.float32)
    hexw = np.asarray(inputs["hex_weights"], dtype=np.float32)
    Wq = np.asarray(inputs["Wq"], dtype=np.float32)
    Wk = np.asarray(inputs["Wk"], dtype=np.float32)
    Wv = np.asarray(inputs["Wv"], dtype=np.float32)
    Wo = np.asarray(inputs["Wo"], dtype=np.float32)
    lam = float(np.asarray(inputs["lam_logit"], dtype=np.float64))
    sig = 1.0 / (1.0 + np.exp(-lam))
    hexgF = np.ascontiguousarray(
        (np.asarray(inputs["hexagrams"], dtype=np.float64)
         * 2.0 * np.sqrt(sig)).astype(np.float16))
    maskT = np.triu(np.full((128, 128), -3.0e38, np.float32), 1)
    maskT = np.ascontiguousarray(maskT.astype(BF16NP))
    identb = np.ascontiguousarray(
        np.concatenate([np.eye(128, dtype=np.float32)] * 2, axis=1)
        .astype(BF16NP))

    in_maps = []
    for c in range(8):
        b, g = c // 4, c % 4
        hs = slice(192 * g, 192 * (g + 1))
        xTn = np.ascontiguousarray(x[b].T.astype(np.float16))
        hexTn = np.ascontiguousarray(hexw[b].T.astype(np.float16))
        wqk = np.concatenate([Wq[hs], Wk[hs]], axis=0)      # [384, 768]
        wqkT = np.ascontiguousarray(wqk.T.astype(np.float16))  # [768, 384]
        wvT = np.ascontiguousarray(Wv[hs].T.astype(np.float16))  # [768, 192]
        woT = np.zeros((256, DM), np.float16)
        woT[:192] = Wo[:, hs].T                             # [192, 768]
        in_maps.append({
            "xT": xTn, "hexT": hexTn, "hexgF": hexgF, "wqkT": wqkT,
            "wvT": wvT, "woT": np.ascontiguousarray(woT),
            "maskT": maskT, "identb": identb,
        })
    return in_maps


LAST_RESULTS = None


def _run(inputs, **kwargs):
    global _CACHED_NC, LAST_RESULTS
    if _CACHED_NC is None:
        _CACHED_NC = _build()
    in_maps = _prep_in_maps(inputs)
    res = run_bass_kernel_spmd(_CACHED_NC, in_maps, core_ids=list(range(8)),
                               **kwargs)
    LAST_RESULTS = res
    outs = [r["out"].astype(np.float32) for r in res.results]
    y = np.empty((2, T, DM), np.float32)
    y[0] = outs[0] + outs[1] + outs[2] + outs[3]
    y[1] = outs[4] + outs[5] + outs[6] + outs[7]
    return y


def kernel(**inputs):
    return _run(inputs)
